# revision 1
# baseline (speedup 1.0000x reference)
"""MoE expert-parallel FFN kernel for Trainium2 (8 NeuronCores).

Problem: 8192 tokens, d_model=768, d_ff=3072, 8 experts; each token is
routed (local_eid) to one expert's FFN: y = (relu(x@W1[e]+b1[e])@W2[e]+b2[e])*gate.

Sharding: expert parallelism — core e gets expert e's weights plus the
(gathered, transposed, zero-padded) tokens routed to expert e. The device
computes, per core, a dense 2-layer FFN in transposed orientation:

    hT[d_ff, T]    = relu(W1.T @ xT + b1)      (lhsT=W1 natural layout)
    yT[d_model, T] = (W2.T @ hT + b2) * gate   (lhsT=W2 natural layout)

so both weights are consumed in their natural [K, M] layouts and biases land
on the partition dim (per-partition activation bias). Gate is broadcast
across partitions once per core. Host side does the gather (by local_eid)
and the scatter-back, which is the all-to-all dispatch of the sharding hint
performed at shard/unshard time.
"""

import numpy as np

import concourse.bacc as bacc
import concourse.mybir as mybir
import concourse.tile as tile
from concourse.bass_utils import run_bass_kernel_spmd

P = 128
D_MODEL = 768
D_FF = 3072
N_EXPERTS = 8
N_CORES = 8
KM1 = D_MODEL // P   # 6  k-tiles for mm1
M1 = D_FF // P       # 24 m-tiles for mm1
KM2 = D_FF // P      # 24 k-tiles for mm2
M2 = D_MODEL // P    # 6  m-tiles for mm2
T_BLOCK_MAX = 1344   # max tokens per on-chip block (SBUF budget)
F32 = mybir.dt.float32


def _chunks(T):
    """Split T into contiguous chunks, each <=512 (fp32 moving-operand max),
    >=256 where possible (full-rate float32r), and a multiple of 8
    (fp32r ISA wants even N; 8 keeps 32B DMA alignment). T must be a
    multiple of 8."""
    assert T % 8 == 0, T
    n = -(-T // 512)
    base = -(-(T // 8) // n) * 8
    out = []
    s = 0
    while s < T:
        e = min(s + base, T)
        out.append((s, e))
        s = e
    return out


def _emit(tc, aps, T, mmdt, reps=1):
    nc = tc.nc
    xT, w1, b1, w2, b2, gate, yT = (
        aps["xT"], aps["w1"], aps["b1"], aps["w2"], aps["b2"], aps["gate"], aps["yT"],
    )
    n_blocks = -(-T // T_BLOCK_MAX)
    TB = -(-(-(-T // n_blocks)) // 8) * 8  # per-block tokens, multiple of 8
    Relu = mybir.ActivationFunctionType.Relu
    # m-tiles of W1 per streamed weight group. Hardware A/B (interleaved,
    # same-session): MG=2 beats MG=4 by ~16us despite the cost model's
    # preference — smaller weight bursts win the DMA arbitration on the
    # PE critical path.
    MG = 2

    with (
        tc.tile_pool(name="const", bufs=1) as const,
        tc.tile_pool(name="xres", bufs=1) as xres,
        tc.tile_pool(name="hres", bufs=1) as hres,
        tc.tile_pool(name="w1s", bufs=2) as w1p,
        tc.tile_pool(name="w2s", bufs=2) as w2p,
        tc.tile_pool(name="ev", bufs=4) as evp,
        tc.tile_pool(name="ps", bufs=2, space="PSUM") as psp,
    ):
        b1_sb = const.tile([P, M1], F32)
        nc.sync.dma_start(out=b1_sb[:], in_=b1.rearrange("(m p) -> p m", p=P))
        b2_sb = const.tile([P, M2], F32)
        nc.sync.dma_start(out=b2_sb[:], in_=b2.rearrange("(m p) -> p m", p=P))

        import contextlib
        loop_cm = tc.For_i(0, reps, 1) if reps != 1 else contextlib.nullcontext()
        with loop_cm:
            _emit_body(
                tc, aps, T, mmdt, n_blocks, TB, Relu, MG,
                const, xres, hres, w1p, w2p, evp, psp, b1_sb, b2_sb,
            )


def _emit_body(tc, aps, T, mmdt, n_blocks, TB, Relu, MG,
               const, xres, hres, w1p, w2p, evp, psp, b1_sb, b2_sb):
    nc = tc.nc
    xT, w1, b1, w2, b2, gate, yT = (
        aps["xT"], aps["w1"], aps["b1"], aps["w2"], aps["b2"], aps["gate"], aps["yT"],
    )

    def mm_cast(ap):
        return ap if ap.dtype == mmdt else ap.bitcast(mmdt)

    if True:
        for blk in range(n_blocks):
            t0 = blk * TB
            t1 = min(T, t0 + TB)
            if t1 <= t0:
                continue
            Tb = t1 - t0
            chs = _chunks(Tb)

            x_all = xres.tile([P, KM1, Tb], mmdt, tag="x")
            xT_r = mm_cast(xT[:, t0:t1].rearrange("(k p) t -> p k t", p=P))
            for k in range(KM1):
                nc.sync.dma_start(out=x_all[:, k], in_=xT_r[:, k])
            hT = hres.tile([P, KM2, Tb], mmdt, tag="h")

            # ---- mm1: hT[:, m, :] = relu(W1[:, mP:(m+1)P].T @ xT + b1[m]) ----
            for mg in range(M1 // MG):
                wt = w1p.tile([P, KM1, MG * P], mmdt, tag="w1")
                w1_r = mm_cast(
                    w1[:, mg * MG * P:(mg + 1) * MG * P]
                    .rearrange("(k p) n -> p k n", p=P)
                )
                for k in range(KM1):
                    nc.sync.dma_start(out=wt[:, k], in_=w1_r[:, k])
                for ms in range(MG):
                    m = mg * MG + ms
                    pst = [
                        psp.tile([P, e - s], F32, tag=f"ps{ci}", name=f"ps{ci}")
                        for ci, (s, e) in enumerate(chs)
                    ]
                    for k in range(KM1):
                        for ci, (s, e) in enumerate(chs):
                            nc.tensor.matmul(
                                pst[ci][:],
                                lhsT=wt[:, k, ms * P:(ms + 1) * P],
                                rhs=x_all[:, k, s:e],
                                start=(k == 0),
                                stop=(k == KM1 - 1),
                            )
                    for ci, (s, e) in enumerate(chs):
                        nc.scalar.activation(
                            hT[:, m, s:e], pst[ci][:], Relu, bias=b1_sb[:, m:m + 1]
                        )

            # ---- mm2: yT[mP:(m+1)P, :] = (W2[:, mP:(m+1)P].T @ hT + b2[m]) * gate ----
            # gate broadcast emitted late: only needed at mm2 evict, keeps the
            # startup DMA window clear for x/W1 (the PE-critical loads).
            gate_sb = xres.tile([P, Tb], F32, tag="g")
            nc.sync.dma_start(out=gate_sb[:], in_=gate[t0:t1].partition_broadcast(P))
            for m in range(M2):
                wt2 = w2p.tile([P, KM2, P], mmdt, tag="w2")
                w2_r = mm_cast(
                    w2[:, m * P:(m + 1) * P]
                    .rearrange("(k p) n -> p k n", p=P)
                )
                for k2 in range(0, KM2, 4):
                    nc.sync.dma_start(out=wt2[:, k2:k2 + 4], in_=w2_r[:, k2:k2 + 4])
                pst = [
                    psp.tile([P, e - s], F32, tag=f"ps{ci}", name=f"ps{ci}")
                    for ci, (s, e) in enumerate(chs)
                ]
                for k in range(KM2):
                    for ci, (s, e) in enumerate(chs):
                        nc.tensor.matmul(
                            pst[ci][:],
                            lhsT=wt2[:, k, :],
                            rhs=hT[:, k, s:e],
                            start=(k == 0),
                            stop=(k == KM2 - 1),
                        )
                for ci, (s, e) in enumerate(chs):
                    yt = evp.tile([P, e - s], F32, tag="y")
                    nc.vector.tensor_scalar_add(yt[:], pst[ci][:], b2_sb[:, m:m + 1])
                    nc.vector.tensor_mul(yt[:], yt[:], gate_sb[:, s:e])
                    nc.sync.dma_start(out=yT[m * P:(m + 1) * P, t0 + s:t0 + e], in_=yt[:])


def build_nc(T, mmdt=mybir.dt.float32r, reps=1):
    nc = bacc.Bacc("TRN2", target_bir_lowering=False, debug=False)
    # 2-byte matmul dtypes get their x/W inputs pre-cast on the host and
    # declared at that dtype in DRAM; 4-byte (fp32/fp32r) keep f32 + bitcast.
    mdt = mmdt if mybir.dt.size(mmdt) == 2 else F32
    aps = {
        "xT": nc.dram_tensor("xT", [D_MODEL, T], mdt, kind="ExternalInput").ap(),
        "w1": nc.dram_tensor("w1", [D_MODEL, D_FF], mdt, kind="ExternalInput").ap(),
        "b1": nc.dram_tensor("b1", [D_FF], F32, kind="ExternalInput").ap(),
        "w2": nc.dram_tensor("w2", [D_FF, D_MODEL], mdt, kind="ExternalInput").ap(),
        "b2": nc.dram_tensor("b2", [D_MODEL], F32, kind="ExternalInput").ap(),
        "gate": nc.dram_tensor("gate", [T], F32, kind="ExternalInput").ap(),
        "yT": nc.dram_tensor("yT", [D_MODEL, T], F32, kind="ExternalOutput").ap(),
    }
    with tile.TileContext(nc) as tc:
        _emit(tc, aps, T, mmdt, reps=reps)
    nc.compile()
    return nc


_NC_CACHE = {}


def _get_nc(T, mmdt):
    key = (T, mmdt)
    if key not in _NC_CACHE:
        _NC_CACHE[key] = build_nc(T, mmdt)
    return _NC_CACHE[key]


def shard_inputs(y_recv, x_flat, gate, local_eid, W1, b1, W2, b2, T_cap,
                 mm_np_dtype=np.float32):
    """Gather tokens per expert, pad to T_cap, transpose. Returns in_maps + idx."""
    eid = np.asarray(local_eid).astype(np.int64)
    in_maps = []
    idxs = []
    for e in range(N_EXPERTS):
        idx = np.nonzero(eid == e)[0]
        idxs.append(idx)
        cnt = len(idx)
        xT = np.zeros((D_MODEL, T_cap), dtype=mm_np_dtype)
        xT[:, :cnt] = np.asarray(x_flat)[idx].T.astype(mm_np_dtype)
        g = np.zeros((T_cap,), dtype=np.float32)
        g[:cnt] = np.asarray(gate)[idx]
        in_maps.append(
            {
                "xT": xT,
                "w1": np.ascontiguousarray(np.asarray(W1)[e], dtype=mm_np_dtype),
                "b1": np.ascontiguousarray(np.asarray(b1)[e], dtype=np.float32),
                "w2": np.ascontiguousarray(np.asarray(W2)[e], dtype=mm_np_dtype),
                "b2": np.ascontiguousarray(np.asarray(b2)[e], dtype=np.float32),
                "gate": g,
            }
        )
    return in_maps, idxs


def t_cap_for(local_eid):
    eid = np.asarray(local_eid).astype(np.int64)
    counts = np.bincount(eid, minlength=N_EXPERTS)
    return max(256, int(-(-int(counts.max()) // 8) * 8))


def kernel(y_recv, x_flat, gate, local_eid, W1, b1, W2, b2, _trace=False):
    T_cap = t_cap_for(local_eid)

    in_maps, idxs = shard_inputs(y_recv, x_flat, gate, local_eid, W1, b1, W2, b2, T_cap)
    nc = _get_nc(T_cap, mybir.dt.float32r)
    res = run_bass_kernel_spmd(
        nc, in_maps, core_ids=list(range(N_CORES)), trace=_trace
    )

    out = np.array(np.asarray(y_recv), dtype=np.float32, copy=True)
    for e in range(N_EXPERTS):
        idx = idxs[e]
        if len(idx):
            out[idx] = res.results[e]["yT"][:, : len(idx)].T
    if _trace:
        return out, res
    return out



# revision 2
# speedup vs baseline: 1.4472x; 1.4472x over previous
"""MoE expert-parallel FFN kernel for Trainium2 (8 NeuronCores).

Problem: 8192 tokens, d_model=768, d_ff=3072, 8 experts; each token is
routed (local_eid) to one expert's FFN: y = (relu(x@W1[e]+b1[e])@W2[e]+b2[e])*gate.

Sharding: expert parallelism — core e gets expert e's weights plus the
(gathered, transposed, zero-padded) tokens routed to expert e. The device
computes, per core, a dense 2-layer FFN in transposed orientation:

    hT[d_ff, T]    = relu(W1.T @ xT + b1)      (lhsT=W1 natural layout)
    yT[d_model, T] = (W2.T @ hT + b2) * gate   (lhsT=W2 natural layout)

so both weights are consumed in their natural [K, M] layouts and biases land
on the partition dim (per-partition activation bias). Gate is broadcast
across partitions once per core. Host side does the gather (by local_eid)
and the scatter-back, which is the all-to-all dispatch of the sharding hint
performed at shard/unshard time.

Matmul operands are bf16 (PE streams 1 col/cycle for both fp32r and bf16, so
compute time is unchanged, but weight DMA bytes halve and FWL engages).
Accumulation stays fp32 in PSUM; rel err ~1e-3 vs the fp32 reference.
Weights are pre-arranged on the host into the SBUF tile layout
([p, group, k, n]) so each weight DMA is a single fully-contiguous
multi-KB-per-partition transfer instead of many 1KB strided lines.
"""

import numpy as np
import ml_dtypes

import concourse.bacc as bacc
import concourse.mybir as mybir
import concourse.tile as tile
from concourse.bass_utils import run_bass_kernel_spmd

P = 128
D_MODEL = 768
D_FF = 3072
N_EXPERTS = 8
N_CORES = 8
KM1 = D_MODEL // P   # 6  k-tiles for mm1
M1 = D_FF // P       # 24 m-tiles for mm1
KM2 = D_FF // P      # 24 k-tiles for mm2
M2 = D_MODEL // P    # 6  m-tiles for mm2
T_BLOCK_MAX = 1344   # max tokens per on-chip block (SBUF budget)
MG = 2               # W1 m-tiles per streamed weight group
F32 = mybir.dt.float32
BF16 = mybir.dt.bfloat16


def _chunks(T):
    """Split T into contiguous chunks, each <=512 (PSUM fp32 bank limit),
    >=256 where possible, and a multiple of 8 (32B DMA alignment).
    T must be a multiple of 8."""
    assert T % 8 == 0, T
    n = -(-T // 512)
    base = -(-(T // 8) // n) * 8
    out = []
    s = 0
    while s < T:
        e = min(s + base, T)
        out.append((s, e))
        s = e
    return out


def _emit(tc, aps, T, mmdt, reps=1):
    nc = tc.nc
    Relu = mybir.ActivationFunctionType.Relu

    with (
        tc.tile_pool(name="const", bufs=1) as const,
        tc.tile_pool(name="xres", bufs=1) as xres,
        tc.tile_pool(name="hres", bufs=1) as hres,
        tc.tile_pool(name="w1s", bufs=2) as w1p,
        tc.tile_pool(name="w2s", bufs=2) as w2p,
        tc.tile_pool(name="ev", bufs=4) as evp,
        tc.tile_pool(name="ps", bufs=2, space="PSUM") as psp,
    ):
        b1_sb = const.tile([P, M1], F32)
        nc.sync.dma_start(out=b1_sb[:], in_=aps["b1"].rearrange("(m p) -> p m", p=P))
        b2_sb = const.tile([P, M2], F32)
        nc.sync.dma_start(out=b2_sb[:], in_=aps["b2"].rearrange("(m p) -> p m", p=P))

        import contextlib
        loop_cm = tc.For_i(0, reps, 1) if reps != 1 else contextlib.nullcontext()
        with loop_cm:
            _emit_body(
                tc, aps, T, mmdt,
                const, xres, hres, w1p, w2p, evp, psp, b1_sb, b2_sb,
            )


def _emit_body(tc, aps, T, mmdt,
               const, xres, hres, w1p, w2p, evp, psp, b1_sb, b2_sb):
    nc = tc.nc
    xT, w1, w2, gate, yT = aps["xT"], aps["w1"], aps["w2"], aps["gate"], aps["yT"]
    Relu = mybir.ActivationFunctionType.Relu

    n_blocks = -(-T // T_BLOCK_MAX)
    TB = -(-(-(-T // n_blocks)) // 8) * 8  # per-block tokens, multiple of 8

    for blk in range(n_blocks):
        t0 = blk * TB
        t1 = min(T, t0 + TB)
        if t1 <= t0:
            continue
        Tb = t1 - t0
        chs = _chunks(Tb)

        x_all = xres.tile([P, KM1, Tb], mmdt, tag="x")
        for k in range(KM1):
            nc.sync.dma_start(out=x_all[:, k], in_=xT[:, k, t0:t1])
        hT = hres.tile([P, KM2, Tb], mmdt, tag="h")

        # ---- mm1: hT[:, m, :] = relu(W1[:, mP:(m+1)P].T @ xT + b1[m]) ----
        for mg in range(M1 // MG):
            wt = w1p.tile([P, KM1, MG * P], mmdt, tag="w1")
            nc.sync.dma_start(out=wt[:], in_=w1[:, mg])
            for ms in range(MG):
                m = mg * MG + ms
                pst = [
                    psp.tile([P, e - s], F32, tag=f"ps{ci}", name=f"ps{ci}")
                    for ci, (s, e) in enumerate(chs)
                ]
                for k in range(KM1):
                    for ci, (s, e) in enumerate(chs):
                        nc.tensor.matmul(
                            pst[ci][:],
                            lhsT=wt[:, k, ms * P:(ms + 1) * P],
                            rhs=x_all[:, k, s:e],
                            start=(k == 0),
                            stop=(k == KM1 - 1),
                        )
                for ci, (s, e) in enumerate(chs):
                    nc.scalar.activation(
                        hT[:, m, s:e], pst[ci][:], Relu, bias=b1_sb[:, m:m + 1]
                    )

        # ---- mm2: yT[:, m, :] = (W2[:, mP:(m+1)P].T @ hT + b2[m]) * gate ----
        # gate broadcast emitted late: only needed at mm2 evict, keeps the
        # startup DMA window clear for x/W1 (the PE-critical loads).
        gate_sb = xres.tile([P, Tb], F32, tag="g")
        nc.sync.dma_start(out=gate_sb[:], in_=gate[t0:t1].partition_broadcast(P))
        for m in range(M2):
            wt2 = w2p.tile([P, KM2, P], mmdt, tag="w2")
            nc.sync.dma_start(out=wt2[:], in_=w2[:, m])
            pst = [
                psp.tile([P, e - s], F32, tag=f"ps{ci}", name=f"ps{ci}")
                for ci, (s, e) in enumerate(chs)
            ]
            for k in range(KM2):
                for ci, (s, e) in enumerate(chs):
                    nc.tensor.matmul(
                        pst[ci][:],
                        lhsT=wt2[:, k, :],
                        rhs=hT[:, k, s:e],
                        start=(k == 0),
                        stop=(k == KM2 - 1),
                    )
            for ci, (s, e) in enumerate(chs):
                yt = evp.tile([P, e - s], F32, tag="y")
                nc.vector.tensor_scalar_add(yt[:], pst[ci][:], b2_sb[:, m:m + 1])
                nc.vector.tensor_mul(yt[:], yt[:], gate_sb[:, s:e])
                nc.sync.dma_start(out=yT[:, m, t0 + s:t0 + e], in_=yt[:])


def build_nc(T, mmdt=BF16, reps=1):
    nc = bacc.Bacc("TRN2", target_bir_lowering=False, debug=False)
    NG1 = M1 // MG
    aps = {
        # xT[p, k, t] = x[t, k*P+p]
        "xT": nc.dram_tensor("xT", [P, KM1, T], mmdt, kind="ExternalInput").ap(),
        # w1[p, g, k, j] = W1[k*P+p, g*MG*P+j]
        "w1": nc.dram_tensor("w1", [P, NG1, KM1, MG * P], mmdt, kind="ExternalInput").ap(),
        "b1": nc.dram_tensor("b1", [D_FF], F32, kind="ExternalInput").ap(),
        # w2[p, m, k, n] = W2[k*P+p, m*P+n]
        "w2": nc.dram_tensor("w2", [P, M2, KM2, P], mmdt, kind="ExternalInput").ap(),
        "b2": nc.dram_tensor("b2", [D_MODEL], F32, kind="ExternalInput").ap(),
        "gate": nc.dram_tensor("gate", [T], F32, kind="ExternalInput").ap(),
        # yT[p, m, t] = y[t, m*P+p]
        "yT": nc.dram_tensor("yT", [P, M2, T], F32, kind="ExternalOutput").ap(),
    }
    with tile.TileContext(nc) as tc:
        _emit(tc, aps, T, mmdt, reps=reps)
    nc.compile()
    return nc


_NC_CACHE = {}


def _get_nc(T, mmdt=BF16):
    key = (T, mmdt)
    if key not in _NC_CACHE:
        _NC_CACHE[key] = build_nc(T, mmdt)
    return _NC_CACHE[key]


def _prearrange_w1(W1e, np_dt):
    # [D_MODEL, D_FF] -> [P, NG1, KM1, MG*P] with w1[p,g,k,j] = W1[k*P+p, g*MG*P+j]
    NG1 = M1 // MG
    return np.ascontiguousarray(
        W1e.reshape(KM1, P, NG1, MG * P).transpose(1, 2, 0, 3).astype(np_dt)
    )


def _prearrange_w2(W2e, np_dt):
    # [D_FF, D_MODEL] -> [P, M2, KM2, P] with w2[p,m,k,n] = W2[k*P+p, m*P+n]
    return np.ascontiguousarray(
        W2e.reshape(KM2, P, M2, P).transpose(1, 2, 0, 3).astype(np_dt)
    )


def shard_inputs(y_recv, x_flat, gate, local_eid, W1, b1, W2, b2, T_cap,
                 mm_np_dtype=ml_dtypes.bfloat16):
    """Gather tokens per expert, pad to T_cap, transpose. Returns in_maps + idx."""
    eid = np.asarray(local_eid).astype(np.int64)
    x_flat = np.asarray(x_flat)
    gate = np.asarray(gate)
    W1 = np.asarray(W1)
    W2 = np.asarray(W2)
    b1 = np.asarray(b1)
    b2 = np.asarray(b2)
    in_maps = []
    idxs = []
    for e in range(N_EXPERTS):
        idx = np.nonzero(eid == e)[0]
        idxs.append(idx)
        cnt = len(idx)
        # xT[p, k, t] = x[t, k*P+p]
        xT = np.zeros((P, KM1, T_cap), dtype=mm_np_dtype)
        xTfull = x_flat[idx].T.reshape(KM1, P, cnt).transpose(1, 0, 2)
        xT[:, :, :cnt] = xTfull.astype(mm_np_dtype)
        g = np.zeros((T_cap,), dtype=np.float32)
        g[:cnt] = gate[idx]
        in_maps.append(
            {
                "xT": xT,
                "w1": _prearrange_w1(W1[e], mm_np_dtype),
                "b1": np.ascontiguousarray(b1[e], dtype=np.float32),
                "w2": _prearrange_w2(W2[e], mm_np_dtype),
                "b2": np.ascontiguousarray(b2[e], dtype=np.float32),
                "gate": g,
            }
        )
    return in_maps, idxs


def t_cap_for(local_eid):
    eid = np.asarray(local_eid).astype(np.int64)
    counts = np.bincount(eid, minlength=N_EXPERTS)
    return max(256, int(-(-int(counts.max()) // 8) * 8))


def kernel(y_recv, x_flat, gate, local_eid, W1, b1, W2, b2, _trace=False):
    T_cap = t_cap_for(local_eid)

    in_maps, idxs = shard_inputs(y_recv, x_flat, gate, local_eid, W1, b1, W2, b2, T_cap)
    nc = _get_nc(T_cap)
    res = run_bass_kernel_spmd(
        nc, in_maps, core_ids=list(range(N_CORES)), trace=_trace
    )

    out = np.array(np.asarray(y_recv), dtype=np.float32, copy=True)
    for e in range(N_EXPERTS):
        idx = idxs[e]
        if len(idx):
            # yT[p, m, t] -> y[t, m*P+p]
            yT = res.results[e]["yT"]  # [P, M2, T_cap]
            y = yT.transpose(2, 1, 0).reshape(T_cap, D_MODEL)
            out[idx] = y[: len(idx)]
    if _trace:
        return out, res
    return out
